# revision 37
# baseline (speedup 1.0000x reference)
"""Trainium2 Bass kernel for nn_CMmodel (retrieval_knn).

Model (per layer, x2):
    sim = cosine(x, mem)                       # [N, 2048]
    S, I = top_k(sim, 10); w = softmax(relu(S))
    h = sum_k w[n,k] * mem[I[n,k]]             # [N, 256]
    h = leaky_relu(batchnorm(h))               # batch stats over ALL N rows

Strategy (8 cores, data-parallel over N):
  - Shard x rows 8 ways; replicate mem banks + BN params; AllReduce the BN
    batch statistics so the global batch stats are exact.
  - Selection must be f32-exact: near-ties between rank 10 and 11 pick
    different mem rows (measured: 1-pass f32r sim flips 0.52% of rows ->
    2.6e-2 rel err; 3-pass flips none). So sim = 3-pass f32r
    (r(x)@r(m^) + res(x)@r(m^) + bf16(x)@bf16res(m^)), threshold and mask
    computed on the f32 q values.
  - Scale-free threshold: q = x @ m^T (mem rows normalized, x NOT
    normalized). Selection is invariant to the positive 1/|x| scale; the
    softmax temperature is folded into the exp activation via
    out = Exp(q*invn - t*invn) with per-partition scale/bias. This kills
    the per-tile sim-drain multiply and all per-tile sqrt work.
  - L1 h-matmul must be full f32 (f32r -> 2.6e-2 rel err via L2 selection
    flips); L2 h-matmul is bf16 (final rel err 2.4e-3, verified in numpy
    simulation of this exact pipeline on the fixed-seed data).
  - All sqrt work (mem norms, x norms, y1 norms, BN) is batched into
    phases so the ACT table ping-pong (Exp vs Sqrt) happens ~4 times
    total instead of per-tile (baseline: 158 table loads, 202us).
  - BN batch stats accumulated per-tile on the otherwise-idle Pool
    engine ([128,512] accumulator), reduced with ONE ones-matmul per
    layer, AllReduced across cores.
  - Emission order avoids ACT head-of-line blocking: the Exp for tile i
    is emitted one iteration later (its DVE threshold chain is done by
    then), and the transpose/h-matmul stage for tile i-2 is emitted
    before it, so no in-order engine queue ever waits on a slow
    cross-engine chain.
"""
import sys

sys.path.insert(0, "/opt/trn_rl_repo")

import numpy as np

import concourse.bacc as bacc
import concourse.mybir as mybir
import concourse.tile as tile
from concourse.bass_utils import run_bass_kernel_spmd
from concourse.masks import make_identity
from concourse.tile import add_dep_helper

F32 = mybir.dt.float32
F32R = mybir.dt.float32r
BF16 = mybir.dt.bfloat16
AF = mybir.ActivationFunctionType
OP = mybir.AluOpType

MEM_DIM = 256
MEM_SIZE = 2048
K_TOP = 10
BN_EPS = 1e-5
LEAKY = 0.01

NJ = MEM_SIZE // 128  # 16 mem-row chunks
NEG_BIG = -1e30
L2_LRELU_EXP = False  # mask via Lrelu(alpha-huge)+Exp on ACT instead of DVE stt


def build_nc(n_cores: int, rows_per_core: int):
    nt = rows_per_core // 128
    n_total = rows_per_core * n_cores
    nc = bacc.Bacc("TRN2", target_bir_lowering=False, debug=False,
                   num_devices=n_cores)

    x_d = nc.dram_tensor("x", [rows_per_core, MEM_DIM], F32, kind="ExternalInput")
    mem_d = {
        1: nc.dram_tensor("mem1", [MEM_SIZE, MEM_DIM], F32, kind="ExternalInput"),
        2: nc.dram_tensor("mem2", [MEM_SIZE, MEM_DIM], F32, kind="ExternalInput"),
    }
    gam_d = {
        1: nc.dram_tensor("gamma1", [1, MEM_DIM], F32, kind="ExternalInput"),
        2: nc.dram_tensor("gamma2", [1, MEM_DIM], F32, kind="ExternalInput"),
    }
    bet_d = {
        1: nc.dram_tensor("beta1", [1, MEM_DIM], F32, kind="ExternalInput"),
        2: nc.dram_tensor("beta2", [1, MEM_DIM], F32, kind="ExternalInput"),
    }
    out_d = nc.dram_tensor("out", [rows_per_core, MEM_DIM], F32, kind="ExternalOutput")
    h_dram = {
        1: nc.dram_tensor("h1buf", [rows_per_core, MEM_DIM], F32),
        2: nc.dram_tensor("h2buf", [rows_per_core, MEM_DIM], F32),
    }

    with tile.TileContext(nc) as tc:
        with tc.tile_pool(name="consts", bufs=1) as consts, \
             tc.tile_pool(name="banks", bufs=1) as banks, \
             tc.tile_pool(name="work", bufs=1) as work, \
             tc.tile_pool(name="psum_q", bufs=1, space="PSUM") as psum_q, \
             tc.tile_pool(name="psum_tp", bufs=3, space="PSUM") as psum_tp, \
             tc.tile_pool(name="psum_h", bufs=1, space="PSUM") as psum_h_pool, \
             tc.tile_pool(name="dram", bufs=1, space="DRAM") as dram:

            # PE emission-order chain: accumulation groups must stay
            # contiguous on PE (interleaved matmuls drop accumulates).
            class _PEChain:
                def __init__(self):
                    self.last = None

                def _chain(self, binst):
                    if self.last is not None:
                        add_dep_helper(binst.ins, self.last.ins, sync=False,
                                       reason="pe-order")
                    self.last = binst
                    return binst

                def matmul(self, *a, **kw):
                    return self._chain(nc.tensor.matmul(*a, **kw))

                def transpose(self, *a, **kw):
                    return self._chain(nc.tensor.transpose(*a, **kw))

            PE = _PEChain()

            # ---------------- constants ----------------
            ident = consts.tile([128, 128], F32)
            make_identity(nc, ident)
            ident_b = consts.tile([128, 128], BF16, name="ident_b")
            nc.vector.tensor_copy(ident_b, ident)
            ones_col = consts.tile([128, 1], F32)
            nc.vector.memset(ones_col, 1.0)
            ones_row = consts.tile([1, 128], F32)
            nc.vector.memset(ones_row, 1.0)
            epsap = consts.tile([1, 1], F32)
            nc.vector.memset(epsap, BN_EPS)

            gb = {}
            for L in (1, 2):
                g = consts.tile([1, MEM_DIM], F32, name=f"gamma_sb{L}")
                b = consts.tile([1, MEM_DIM], F32, name=f"beta_sb{L}")
                nc.sync.dma_start(g, gam_d[L][:])
                nc.sync.dma_start(b, bet_d[L][:])
                gb[L] = (g, b)

            # persistent stores
            # mnT_r[k]: normalized mem transposed, f32r   [128, 2048] x2
            # mnTres[k]: bf16 residual of mnT              [128, 2048] x2
            # (re-prepped between layers: L2 overwrites L1's tiles)
            mnT_r = [banks.tile([128, MEM_SIZE], F32R, name=f"mnTr{k}")
                     for k in range(2)]
            mnTres = [banks.tile([128, MEM_SIZE], BF16, name=f"mnTres{k}")
                      for k in range(2)]
            mraw1 = banks.tile([128, NJ * MEM_DIM], F32, name="mraw1")
            mraw2 = banks.tile([128, NJ * MEM_DIM], BF16, name="mraw2")
            # transposed lhs source (f32): x^T for L1, then y1^T for L2
            xT_f = banks.tile([128, nt * MEM_DIM], F32, name="xT_f")
            # per-row norms^2 / inv-norms, one column per tile
            xns_all = consts.tile([128, nt], F32, name="xns_all")
            invn_all = consts.tile([128, nt], F32, name="invn_all")
            nrm2_all = consts.tile([128, NJ], F32, name="nrm2_all")
            # BN broadcast affine params (row layout)
            a2b = consts.tile([128, MEM_DIM], F32, name="a2b")
            b2b = consts.tile([128, MEM_DIM], F32, name="b2b")
            # BN stats accumulator: cols 0:256 sum(h), 256:512 sum(h^2)
            acc = consts.tile([128, 2 * MEM_DIM], F32, name="acc")

            # ---------------- mem bank prep ----------------
            def prep_bank_a(L):
                """Load raw mem (f32 for L1 h-matmul, bf16 for L2), row
                norms^2. DMAs ride the ACT queue so they never sit behind a
                collective-blocked Sync DMA."""
                md = mem_d[L]
                for j in range(NJ):
                    mr = work.tile([128, MEM_DIM], F32, tag="ld", name="ld", bufs=3)
                    nc.scalar.dma_start(mr, md[j * 128:(j + 1) * 128, :])
                    if L == 1:
                        nc.scalar.copy(mraw1[:, j * MEM_DIM:(j + 1) * MEM_DIM], mr)
                    else:
                        nc.vector.tensor_copy(
                            mraw2[:, j * MEM_DIM:(j + 1) * MEM_DIM], mr)
                    sq = work.tile([128, MEM_DIM], F32, tag="sqt", name="sqt", bufs=2)
                    nc.scalar.activation(sq, mr, AF.Square,
                                         accum_out=nrm2_all[:, j:j + 1])

            def prep_bank_b(L):
                """Batched rsqrt + Newton, then normalize/transpose/split."""
                md = mem_d[L]
                nrm = work.tile([128, NJ], F32, tag="nrm", name="nrm", bufs=1)
                nc.scalar.activation(nrm, nrm2_all, AF.Sqrt)
                inm0 = work.tile([128, NJ], F32, tag="inm0", name="inm0", bufs=1)
                nc.vector.reciprocal(inm0, nrm)
                t1 = work.tile([128, NJ], F32, tag="nt1", name="nt1", bufs=1)
                nc.vector.tensor_mul(t1, inm0, inm0)
                nc.vector.tensor_mul(t1, t1, nrm2_all)
                nc.vector.tensor_scalar(t1, t1, -0.5, 1.5, op0=OP.mult, op1=OP.add)
                inm = work.tile([128, NJ], F32, tag="inm", name="inm", bufs=1)
                nc.vector.tensor_mul(inm, inm0, t1)
                # second pass: scale + transpose + split. Always normalize
                # from a fresh f32 DMA of mem (mraw2 is bf16-rounded; the
                # f32r sim banks must be derived from exact f32 mem or the
                # top-10 selection flips).
                for j in range(NJ):
                    mr2 = work.tile([128, MEM_DIM], F32, tag="ld", name="ld5", bufs=3)
                    nc.scalar.dma_start(mr2, md[j * 128:(j + 1) * 128, :])
                    mnsc = work.tile([128, MEM_DIM], F32, tag="mnsc", name="mnsc",
                                     bufs=2)
                    nc.scalar.mul(mnsc, mr2, inm[:, j:j + 1])
                    tp = psum_tp.tile([128, MEM_DIM], F32, tag="tp")
                    for k in range(2):
                        PE.transpose(tp[:, k * 128:(k + 1) * 128],
                                     mnsc[:, k * 128:(k + 1) * 128], ident)
                    for k in range(2):
                        dstT = mnT_r[k][:, j * 128:(j + 1) * 128]
                        nc.scalar.copy(dstT, tp[:, k * 128:(k + 1) * 128])
                        nc.vector.scalar_tensor_tensor(
                            out=mnTres[k][:, j * 128:(j + 1) * 128],
                            in0=tp[:, k * 128:(k + 1) * 128], scalar=0.0,
                            in1=dstT.bitcast(F32),
                            op0=OP.add, op1=OP.subtract)

            # ---------------- x pre-pass ----------------
            def prep_x():
                for i in range(nt):
                    xi = work.tile([128, MEM_DIM], F32, tag="ld", name="ld2", bufs=3)
                    nc.sync.dma_start(xi, x_d[i * 128:(i + 1) * 128, :])
                    sq = work.tile([128, MEM_DIM], F32, tag="sqt", name="sqt2", bufs=2)
                    nc.scalar.activation(sq, xi, AF.Square,
                                         accum_out=xns_all[:, i:i + 1])
                    tp = psum_tp.tile([128, MEM_DIM], F32, tag="tp")
                    for k in range(2):
                        PE.transpose(tp[:, k * 128:(k + 1) * 128],
                                     xi[:, k * 128:(k + 1) * 128], ident)
                    nc.scalar.copy(xT_f[:, i * MEM_DIM:(i + 1) * MEM_DIM], tp)
                xnr = work.tile([128, nt], F32, tag="xnr", name="xnr", bufs=1)
                nc.scalar.activation(xnr, xns_all, AF.Sqrt)
                nc.vector.reciprocal(invn_all, xnr)

            # ---------------- main loop pieces ----------------
            def stage_sim(L, i):
                """Derive f32r/bf16 lhs splits for tile i, run the 3-pass sim
                matmul into one 4-bank PSUM tile, drain to f32 SBUF."""
                # lhs splits all on the Pool engine (same f32r encoding as
                # ACT, verified bit-identical) — keeps ACT/DVE for the wide
                # per-tile work
                src = xT_f[:, i * MEM_DIM:(i + 1) * MEM_DIM]
                xr = work.tile([128, MEM_DIM], F32R, tag="xr", name="xr", bufs=2)
                nc.gpsimd.tensor_copy(xr, src)
                rsd = work.tile([128, MEM_DIM], F32, tag="rsdl", name="rsdl", bufs=2)
                nc.gpsimd.tensor_sub(rsd, src, xr.bitcast(F32))
                xs = work.tile([128, MEM_DIM], F32R, tag="xs", name="xs", bufs=2)
                nc.gpsimd.tensor_copy(xs, rsd)
                xb = work.tile([128, MEM_DIM], BF16, tag="xb", name="xb", bufs=2)
                nc.gpsimd.tensor_copy(xb, xr.bitcast(F32))

                q = psum_q.tile([128, MEM_SIZE], F32, tag="q")
                for c in range(4):
                    sl = slice(c * 512, (c + 1) * 512)
                    for k in range(2):
                        PE.matmul(q[:, sl], xr[:, k * 128:(k + 1) * 128],
                                  mnT_r[k][:, sl], start=(k == 0), stop=False)
                    for k in range(2):
                        PE.matmul(q[:, sl], xs[:, k * 128:(k + 1) * 128],
                                  mnT_r[k][:, sl], start=False, stop=False)
                    for k in range(2):
                        PE.matmul(q[:, sl], xb[:, k * 128:(k + 1) * 128],
                                  mnTres[k][:, sl], start=False, stop=(k == 1))
                # drain in 4 chunks: each starts as soon as its PSUM bank's
                # accumulation group closes, so the drain overlaps the rest
                # of the sim and the q banks free up chunk-by-chunk.
                # L1: the two LATE chunks go to DVE — they feed DVE's own
                # max8 (which waits for them anyway) and keep the ACT queue
                # clear at the iteration boundary so the next tile's
                # transpose PSUM reuse isn't blocked. L2: DVE is saturated
                # by scans+stt, so all four stay on ACT.
                s_sb = work.tile([128, MEM_SIZE], F32, tag="s_sb", name="s_sb", bufs=2)
                for c in range(4):
                    sl = slice(c * 512, (c + 1) * 512)
                    if L == 1 and c >= 2:
                        nc.vector.tensor_copy(s_sb[:, sl], q[:, sl])
                    else:
                        nc.scalar.copy(s_sb[:, sl], q[:, sl])
                return s_sb

            def stage_scan(L, i, s_sb):
                """Exact 10th-largest threshold of tile i (3 DVE passes),
                plus the exp bias -t/|x| so next iteration's ACT work has
                no DVE wait."""
                m8a = work.tile([128, 8], F32, tag="m8a", name="m8a", bufs=2)
                nc.vector.max(out=m8a, in_=s_sb)
                sz = work.tile([128, MEM_SIZE], F32, tag="sz", name="sz", bufs=2)
                nc.vector.match_replace(out=sz, in_to_replace=m8a,
                                        in_values=s_sb, imm_value=NEG_BIG)
                m8b = work.tile([128, 8], F32, tag="m8b", name="m8b", bufs=2)
                nc.vector.max(out=m8b, in_=sz)
                t_ap = m8b[:, K_TOP - 8 - 1:K_TOP - 8]  # 10th largest
                invn_col = invn_all[:, i:i + 1]
                negt = work.tile([128, 1], F32, tag="negt", name="negt", bufs=2)
                nc.gpsimd.tensor_scalar(negt, t_ap, -1.0, None, op0=OP.mult)
                negb = work.tile([128, 1], F32, tag="negb", name="negb", bufs=2)
                nc.gpsimd.tensor_mul(negb, negt, invn_col)
                return sz, m8b, negb

            def stage_weights(L, i, s_sb, sz, m8b, negb):
                """exp weights, mask, Z for tile i (emitted early in the
                next iteration: all inputs are ready, so neither the ACT nor
                the DVE queue ever stalls on a cross-engine chain).

                L1: Exp on ACT + mask/Z via one DVE stt (DVE has slack).
                L2: mask folded into ACT entirely: v = Lrelu(arg, alpha=1e7)
                sends unselected entries to huge-negative, Exp underflows
                them to exactly 0; selected entries pass through unchanged.
                Z comes free from the Exp's accumulator. DVE never touches
                the 2048-wide weights in L2."""
                t_ap = m8b[:, K_TOP - 8 - 1:K_TOP - 8]  # 10th largest
                invn_col = invn_all[:, i:i + 1]
                Z = work.tile([128, 1], F32, tag="Z", name="Z", bufs=2)
                if L == 1:
                    # e written into sz (dead after its max8)
                    nc.scalar.activation(sz, s_sb, AF.Exp, bias=negb,
                                         scale=invn_col)
                    U = work.tile([128, MEM_SIZE], F32, tag="U", name="U", bufs=2)
                    nc.vector.scalar_tensor_tensor(
                        out=U, in0=s_sb, scalar=t_ap, in1=sz,
                        op0=OP.is_ge, op1=OP.mult, accum_out=Z)
                elif L2_LRELU_EXP:
                    nc.scalar.activation(sz, s_sb, AF.Lrelu, bias=negb,
                                         scale=invn_col, alpha=1e7)
                    U = work.tile([128, MEM_SIZE], BF16, tag="U", name="U", bufs=2)
                    nc.scalar.activation(U, sz, AF.Exp, accum_out=Z)
                else:
                    nc.scalar.activation(sz, s_sb, AF.Exp, bias=negb,
                                         scale=invn_col)
                    U = work.tile([128, MEM_SIZE], BF16, tag="U", name="U", bufs=2)
                    nc.vector.scalar_tensor_tensor(
                        out=U, in0=s_sb, scalar=t_ap, in1=sz,
                        op0=OP.is_ge, op1=OP.mult, accum_out=Z)
                rz = work.tile([128, 1], F32, tag="rz", name="rz", bufs=2)
                nc.vector.reciprocal(rz, Z)
                return dict(U=U, rz=rz)

            def stage2(L, i, st):
                """U transposes + h = (U/Z) @ mem + Pool stats accumulate."""
                U, rz = st["U"], st["rz"]
                ut_dt = F32 if L == 1 else BF16
                idn = ident if L == 1 else ident_b
                utp = work.tile([128, MEM_SIZE], ut_dt, tag="utp", name="utp", bufs=1)
                for g in range(4):
                    tp = psum_tp.tile([128, 512], ut_dt, tag="tp")
                    for c4 in range(4):
                        c = g * 4 + c4
                        PE.transpose(tp[:, c4 * 128:(c4 + 1) * 128],
                                     U[:, c * 128:(c + 1) * 128], idn)
                    if L == 1 and g == 3:
                        nc.vector.tensor_copy(utp[:, g * 512:(g + 1) * 512], tp)
                    else:
                        nc.scalar.copy(utp[:, g * 512:(g + 1) * 512], tp)
                mraw = mraw1 if L == 1 else mraw2
                hp = psum_h_pool.tile([128, MEM_DIM], F32, tag="hp")
                for c in range(NJ):
                    PE.matmul(hp, utp[:, c * 128:(c + 1) * 128],
                              mraw[:, c * MEM_DIM:(c + 1) * MEM_DIM],
                              start=(c == 0), stop=(c == NJ - 1))
                dst = work.tile([128, MEM_DIM], F32, tag="dst", name="dst", bufs=3)
                nc.scalar.mul(dst, hp, rz)
                nc.sync.dma_start(h_dram[L][i * 128:(i + 1) * 128, :], dst)
                nc.gpsimd.tensor_add(acc[:, 0:MEM_DIM], acc[:, 0:MEM_DIM], dst)
                sqh = work.tile([128, MEM_DIM], F32, tag="sqh", name="sqh", bufs=2)
                nc.gpsimd.tensor_mul(sqh, dst, dst)
                nc.gpsimd.tensor_add(acc[:, MEM_DIM:2 * MEM_DIM],
                                     acc[:, MEM_DIM:2 * MEM_DIM], sqh)

            def layer(L):
                """Software-pipelined main loop.

                Iteration i emits: sim_i + drain_i; stage2(i-2);
                scans_i; weights_{i-1}. This keeps every in-order engine
                queue free of waits on slow cross-engine chains.
                """
                nc.gpsimd.memset(acc, 0.0)
                scanned = {}   # i -> (s_sb, sz, m8b, negb)
                ready = {}     # i -> st dict (U, rz)
                for i in range(nt):
                    if i >= 2:
                        stage2(L, i - 2, ready.pop(i - 2))
                    if i >= 1:
                        ready[i - 1] = stage_weights(L, i - 1,
                                                     *scanned.pop(i - 1))
                    s_sb = stage_sim(L, i)
                    scanned[i] = (s_sb,) + stage_scan(L, i, s_sb)
                # epilogue
                ready[nt - 1] = stage_weights(L, nt - 1, *scanned.pop(nt - 1))
                stage2(L, nt - 2, ready.pop(nt - 2))
                stage2(L, nt - 1, ready.pop(nt - 1))

            def bn_start(L):
                """Reduce the Pool-accumulated stats and kick the AllReduce.
                Returns the SBUF tile that will hold the global sums."""
                st_ps = psum_h_pool.tile([1, 2 * MEM_DIM], F32, tag="hp")
                PE.matmul(st_ps, ones_col, acc, start=True, stop=True)
                stats_sb = work.tile([1, 2 * MEM_DIM], F32, tag="stats",
                                     name="stats", bufs=1)
                nc.scalar.copy(stats_sb, st_ps)
                ar_in = dram.tile([1, 2 * MEM_DIM], F32, name=f"ar_in{L}")
                ar_out = dram.tile([1, 2 * MEM_DIM], F32, addr_space="Shared",
                                   name=f"ar_out{L}")
                nc.sync.dma_start(ar_in, stats_sb)
                nc.gpsimd.collective_compute(
                    "AllReduce", OP.add,
                    replica_groups=[list(range(n_cores))],
                    ins=[ar_in[:]], outs=[ar_out[:]],
                )
                gst = work.tile([1, 2 * MEM_DIM], F32, tag="gst", name="gst", bufs=1)
                nc.sync.dma_start(gst, ar_out)
                return gst

            def bn_finish(L, gst):
                """Turn global sums into broadcast affine params a2b/b2b."""
                gamma_sb, beta_sb = gb[L]
                ab = work.tile([1, 2 * MEM_DIM], F32, tag="ab", name="ab", bufs=1)
                a_ap, b_ap = ab[:, 0:MEM_DIM], ab[:, MEM_DIM:2 * MEM_DIM]
                mu = work.tile([1, MEM_DIM], F32, tag="mu", name="mu", bufs=1)
                nc.vector.tensor_scalar(mu, gst[:, 0:MEM_DIM], 1.0 / n_total,
                                        None, op0=OP.mult)
                ex2 = work.tile([1, MEM_DIM], F32, tag="ex2", name="ex2", bufs=1)
                nc.vector.tensor_scalar(ex2, gst[:, MEM_DIM:2 * MEM_DIM],
                                        1.0 / n_total, None, op0=OP.mult)
                musq = work.tile([1, MEM_DIM], F32, tag="musq", name="musq", bufs=1)
                nc.scalar.activation(musq, mu, AF.Square)
                var = work.tile([1, MEM_DIM], F32, tag="var", name="var", bufs=1)
                nc.vector.tensor_sub(var, ex2, musq)
                sd = work.tile([1, MEM_DIM], F32, tag="sd", name="sd", bufs=1)
                nc.scalar.activation(sd, var, AF.Sqrt, bias=epsap)
                isd = work.tile([1, MEM_DIM], F32, tag="isd", name="isd", bufs=1)
                nc.vector.reciprocal(isd, sd)
                nc.vector.tensor_mul(a_ap, gamma_sb, isd)
                mua = work.tile([1, MEM_DIM], F32, tag="mua", name="mua", bufs=1)
                nc.vector.tensor_mul(mua, mu, a_ap)
                nc.vector.tensor_sub(b_ap, beta_sb, mua)
                bc = psum_tp.tile([128, 2 * MEM_DIM], F32, tag="tp")
                PE.matmul(bc, ones_row, ab, start=True, stop=True)
                nc.scalar.copy(a2b, bc[:, 0:MEM_DIM])
                nc.scalar.copy(b2b, bc[:, MEM_DIM:2 * MEM_DIM])

            def prefetch_h(L, n_pre):
                """Kick the first h-tile DMA loads before the BN collective
                blocks the queues. Returns the load tiles."""
                tiles = []
                for i in range(n_pre):
                    hsl = work.tile([128, MEM_DIM], F32, tag=f"pre{i}",
                                    name=f"pre{L}_{i}", bufs=1)
                    nc.sync.dma_start(hsl, h_dram[L][i * 128:(i + 1) * 128, :])
                    tiles.append(hsl)
                return tiles

            N_PRE = 8

            def prep_y1(pre):
                """Apply BN1 affine + lrelu to h1 tiles (row layout), compute
                row norms, store y1 transposed (f32) over xT_f."""
                for i in range(nt):
                    if i < len(pre):
                        hsl = pre[i]
                    else:
                        hsl = work.tile([128, MEM_DIM], F32, tag="ld",
                                        name="ld3", bufs=3)
                        nc.scalar.dma_start(hsl,
                                            h_dram[1][i * 128:(i + 1) * 128, :])
                    ya = work.tile([128, MEM_DIM], F32, tag="ya", name="ya", bufs=2)
                    nc.gpsimd.tensor_mul(ya, hsl, a2b)
                    yb = work.tile([128, MEM_DIM], F32, tag="yb", name="yb", bufs=2)
                    nc.gpsimd.tensor_add(yb, ya, b2b)
                    ylr = work.tile([128, MEM_DIM], F32, tag="ylr", name="ylr", bufs=2)
                    nc.scalar.activation(ylr, yb, AF.Lrelu, alpha=LEAKY)
                    sq = work.tile([128, MEM_DIM], F32, tag="sqt", name="sqt3", bufs=2)
                    nc.scalar.activation(sq, ylr, AF.Square,
                                         accum_out=xns_all[:, i:i + 1])
                    tp = psum_tp.tile([128, MEM_DIM], F32, tag="tp")
                    for k in range(2):
                        PE.transpose(tp[:, k * 128:(k + 1) * 128],
                                     ylr[:, k * 128:(k + 1) * 128], ident)
                    nc.scalar.copy(xT_f[:, i * MEM_DIM:(i + 1) * MEM_DIM], tp)
                xnr = work.tile([128, nt], F32, tag="xnr", name="xnr2", bufs=1)
                nc.scalar.activation(xnr, xns_all, AF.Sqrt)
                nc.vector.reciprocal(invn_all, xnr)

            def final_out(pre):
                for i in range(nt):
                    if i < len(pre):
                        hsl = pre[i]
                    else:
                        hsl = work.tile([128, MEM_DIM], F32, tag="ld",
                                        name="ld4", bufs=3)
                        nc.scalar.dma_start(hsl,
                                            h_dram[2][i * 128:(i + 1) * 128, :])
                    eng = nc.gpsimd if i % 2 == 0 else nc.vector
                    ya = work.tile([128, MEM_DIM], F32, tag="ya", name="ya2", bufs=2)
                    eng.tensor_mul(ya, hsl, a2b)
                    yb = work.tile([128, MEM_DIM], F32, tag="yb", name="yb2", bufs=2)
                    eng.tensor_add(yb, ya, b2b)
                    yo = work.tile([128, MEM_DIM], F32, tag="ylr", name="yo", bufs=2)
                    nc.scalar.activation(yo, yb, AF.Lrelu, alpha=LEAKY)
                    nc.sync.dma_start(out_d[i * 128:(i + 1) * 128, :], yo)

            # ---------------- program ----------------
            prep_x()
            prep_bank_a(1)
            prep_bank_b(1)
            layer(1)
            pre1 = prefetch_h(1, N_PRE)
            prep_bank_a(2)     # L2 bank prep overlaps the L1 pipeline
            prep_bank_b(2)     # drain-down and the AllReduce latency
            gst1 = bn_start(1)
            bn_finish(1, gst1)
            prep_y1(pre1)
            layer(2)
            pre2 = prefetch_h(2, N_PRE)
            gst2 = bn_start(2)
            bn_finish(2, gst2)
            final_out(pre2)

    nc.compile()
    return nc


_CACHE = {}


def _get_nc(n_cores, rows_per_core):
    key = (n_cores, rows_per_core)
    if key not in _CACHE:
        _CACHE[key] = build_nc(n_cores, rows_per_core)
    return _CACHE[key]


def kernel(x, mem1, mem2, gamma1, beta1, gamma2, beta2, _trace=False,
           _use_f32r=True, _n_cores=8):
    n_cores = _n_cores
    n, d = x.shape
    rows_per_core = n // n_cores
    nc = _get_nc(n_cores, rows_per_core)

    in_maps = []
    for c in range(n_cores):
        in_maps.append({
            "x": np.ascontiguousarray(x[c * rows_per_core:(c + 1) * rows_per_core]),
            "mem1": np.ascontiguousarray(mem1),
            "mem2": np.ascontiguousarray(mem2),
            "gamma1": np.ascontiguousarray(gamma1.reshape(1, -1)),
            "beta1": np.ascontiguousarray(beta1.reshape(1, -1)),
            "gamma2": np.ascontiguousarray(gamma2.reshape(1, -1)),
            "beta2": np.ascontiguousarray(beta2.reshape(1, -1)),
        })
    res = run_bass_kernel_spmd(nc, in_maps, list(range(n_cores)), trace=_trace)
    out = np.concatenate([res.results[c]["out"] for c in range(n_cores)], axis=0)
    if _trace:
        return out, res
    return out


# revision 38
# speedup vs baseline: 1.0261x; 1.0261x over previous
"""Trainium2 Bass kernel for nn_CMmodel (retrieval_knn).

Model (per layer, x2):
    sim = cosine(x, mem)                       # [N, 2048]
    S, I = top_k(sim, 10); w = softmax(relu(S))
    h = sum_k w[n,k] * mem[I[n,k]]             # [N, 256]
    h = leaky_relu(batchnorm(h))               # batch stats over ALL N rows

Strategy (8 cores, data-parallel over N):
  - Shard x rows 8 ways; replicate mem banks + BN params; AllReduce the BN
    batch statistics so the global batch stats are exact.
  - Selection must be f32-exact: near-ties between rank 10 and 11 pick
    different mem rows (measured: 1-pass f32r sim flips 0.52% of rows ->
    2.6e-2 rel err; 3-pass flips none). So sim = 3-pass f32r
    (r(x)@r(m^) + res(x)@r(m^) + bf16(x)@bf16res(m^)), threshold and mask
    computed on the f32 q values.
  - Scale-free threshold: q = x @ m^T (mem rows normalized, x NOT
    normalized). Selection is invariant to the positive 1/|x| scale; the
    softmax temperature is folded into the exp activation via
    out = Exp(q*invn - t*invn) with per-partition scale/bias.
  - L1 h-matmul must be full f32 (f32r -> 2.6e-2 rel err via L2 selection
    flips); L2 h-matmul is bf16 (final rel err 2.4e-3, verified in numpy
    simulation of this exact pipeline on the fixed-seed data).
  - NO sqrt anywhere: every 1/sqrt becomes Exp(-0.5*Ln(.)) - Ln, Exp,
    Square, Copy and parametric_relu all live in ONE ACT table
    (natural_log_exp_and_others), so there are zero mid-kernel table
    reloads (the old Sqrt<->Exp ping-pong cost 202us in the original
    baseline) AND the x/y1 norm work can fuse into the main loops.
  - Everything is ONE software-pipelined loop per layer: per-tile lhs
    prep (DMA, BN-apply for L2, transpose, f32r splits, row norms) runs
    one iteration ahead; the threshold scans run one iteration behind
    the sim; exp/mask one more behind; transpose+h-matmul two behind.
    Engine assignment is balanced per layer (L1 is PE-bound, L2 is
    PE/DVE/ACT-balanced) and in-order queues never wait on a slow
    cross-engine chain.
  - BN batch stats accumulated per-tile on the Pool engine, reduced with
    ONE ones-matmul per layer, AllReduced across cores.
"""
import sys

sys.path.insert(0, "/opt/trn_rl_repo")

import numpy as np

import concourse.bacc as bacc
import concourse.mybir as mybir
import concourse.tile as tile
from concourse.bass_utils import run_bass_kernel_spmd
from concourse.masks import make_identity
from concourse.tile import add_dep_helper

F32 = mybir.dt.float32
F32R = mybir.dt.float32r
BF16 = mybir.dt.bfloat16
AF = mybir.ActivationFunctionType
OP = mybir.AluOpType

MEM_DIM = 256
MEM_SIZE = 2048
K_TOP = 10
BN_EPS = 1e-5
LEAKY = 0.01

NJ = MEM_SIZE // 128  # 16 mem-row chunks
NEG_BIG = -1e30


def build_nc(n_cores: int, rows_per_core: int):
    nt = rows_per_core // 128
    n_total = rows_per_core * n_cores
    nc = bacc.Bacc("TRN2", target_bir_lowering=False, debug=False,
                   num_devices=n_cores)

    x_d = nc.dram_tensor("x", [rows_per_core, MEM_DIM], F32, kind="ExternalInput")
    mem_d = {
        1: nc.dram_tensor("mem1", [MEM_SIZE, MEM_DIM], F32, kind="ExternalInput"),
        2: nc.dram_tensor("mem2", [MEM_SIZE, MEM_DIM], F32, kind="ExternalInput"),
    }
    gam_d = {
        1: nc.dram_tensor("gamma1", [1, MEM_DIM], F32, kind="ExternalInput"),
        2: nc.dram_tensor("gamma2", [1, MEM_DIM], F32, kind="ExternalInput"),
    }
    bet_d = {
        1: nc.dram_tensor("beta1", [1, MEM_DIM], F32, kind="ExternalInput"),
        2: nc.dram_tensor("beta2", [1, MEM_DIM], F32, kind="ExternalInput"),
    }
    out_d = nc.dram_tensor("out", [rows_per_core, MEM_DIM], F32, kind="ExternalOutput")
    h_dram = {
        1: nc.dram_tensor("h1buf", [rows_per_core, MEM_DIM], F32),
        2: nc.dram_tensor("h2buf", [rows_per_core, MEM_DIM], F32),
    }

    with tile.TileContext(nc) as tc:
        with tc.tile_pool(name="consts", bufs=1) as consts, \
             tc.tile_pool(name="banks", bufs=1) as banks, \
             tc.tile_pool(name="work", bufs=1) as work, \
             tc.tile_pool(name="psum_q", bufs=1, space="PSUM") as psum_q, \
             tc.tile_pool(name="psum_tp", bufs=3, space="PSUM") as psum_tp, \
             tc.tile_pool(name="psum_h", bufs=1, space="PSUM") as psum_h_pool, \
             tc.tile_pool(name="dram", bufs=1, space="DRAM") as dram:

            # PE emission-order chain: accumulation groups must stay
            # contiguous on PE (interleaved matmuls drop accumulates).
            class _PEChain:
                def __init__(self):
                    self.last = None

                def _chain(self, binst):
                    if self.last is not None:
                        add_dep_helper(binst.ins, self.last.ins, sync=False,
                                       reason="pe-order")
                    self.last = binst
                    return binst

                def matmul(self, *a, **kw):
                    return self._chain(nc.tensor.matmul(*a, **kw))

                def transpose(self, *a, **kw):
                    return self._chain(nc.tensor.transpose(*a, **kw))

            PE = _PEChain()

            # ---------------- constants ----------------
            ident = consts.tile([128, 128], F32)
            make_identity(nc, ident)
            ident_b = consts.tile([128, 128], BF16, name="ident_b")
            nc.vector.tensor_copy(ident_b, ident)
            ones_col = consts.tile([128, 1], F32)
            nc.vector.memset(ones_col, 1.0)
            ones_row = consts.tile([1, 128], F32)
            nc.vector.memset(ones_row, 1.0)
            epsap = consts.tile([1, 1], F32)
            nc.vector.memset(epsap, BN_EPS)

            gb = {}
            for L in (1, 2):
                g = consts.tile([1, MEM_DIM], F32, name=f"gamma_sb{L}")
                b = consts.tile([1, MEM_DIM], F32, name=f"beta_sb{L}")
                nc.sync.dma_start(g, gam_d[L][:])
                nc.sync.dma_start(b, bet_d[L][:])
                gb[L] = (g, b)

            # persistent stores
            mnT_r = [banks.tile([128, MEM_SIZE], F32R, name=f"mnTr{k}")
                     for k in range(2)]
            mnTres = [banks.tile([128, MEM_SIZE], BF16, name=f"mnTres{k}")
                      for k in range(2)]
            mraw1 = banks.tile([128, NJ * MEM_DIM], F32, name="mraw1")
            mraw2 = banks.tile([128, NJ * MEM_DIM], BF16, name="mraw2")
            nrm2_all = consts.tile([128, NJ], F32, name="nrm2_all")
            # BN broadcast affine params (row layout)
            a2b = consts.tile([128, MEM_DIM], F32, name="a2b")
            b2b = consts.tile([128, MEM_DIM], F32, name="b2b")
            # BN stats accumulator: cols 0:256 sum(h), 256:512 sum(h^2)
            acc = consts.tile([128, 2 * MEM_DIM], F32, name="acc")

            # ---------------- mem bank prep ----------------
            def prep_bank_a(L):
                """Load raw mem (f32 for L1 h-matmul, bf16 for L2) + row
                norms^2. DMAs ride the ACT queue so they never sit behind a
                collective-blocked Sync DMA."""
                md = mem_d[L]
                for j in range(NJ):
                    mr = work.tile([128, MEM_DIM], F32, tag="ld", name="ld", bufs=3)
                    nc.scalar.dma_start(mr, md[j * 128:(j + 1) * 128, :])
                    if L == 1:
                        nc.vector.tensor_copy(
                            mraw1[:, j * MEM_DIM:(j + 1) * MEM_DIM], mr)
                    else:
                        nc.vector.tensor_copy(
                            mraw2[:, j * MEM_DIM:(j + 1) * MEM_DIM], mr)
                    sq = work.tile([128, MEM_DIM], F32, tag="sqt", name="sqt", bufs=2)
                    nc.scalar.activation(sq, mr, AF.Square,
                                         accum_out=nrm2_all[:, j:j + 1])

            def prep_bank_b(L):
                """inm = exp(-0.5 ln(|m|^2)); normalize/transpose/split."""
                md = mem_d[L]
                lnm = work.tile([128, NJ], F32, tag="lnm", name="lnm", bufs=1)
                nc.scalar.activation(lnm, nrm2_all, AF.Ln)
                inm = work.tile([128, NJ], F32, tag="inm", name="inm", bufs=1)
                nc.scalar.activation(inm, lnm, AF.Exp, scale=-0.5)
                for j in range(NJ):
                    mr2 = work.tile([128, MEM_DIM], F32, tag="ld", name="ld5", bufs=3)
                    nc.scalar.dma_start(mr2, md[j * 128:(j + 1) * 128, :])
                    mnsc = work.tile([128, MEM_DIM], F32, tag="mnsc", name="mnsc",
                                     bufs=2)
                    nc.scalar.mul(mnsc, mr2, inm[:, j:j + 1])
                    tp = psum_tp.tile([128, MEM_DIM], F32, tag="tp")
                    for k in range(2):
                        PE.transpose(tp[:, k * 128:(k + 1) * 128],
                                     mnsc[:, k * 128:(k + 1) * 128], ident)
                    for k in range(2):
                        dstT = mnT_r[k][:, j * 128:(j + 1) * 128]
                        nc.scalar.copy(dstT, tp[:, k * 128:(k + 1) * 128])
                        nc.vector.scalar_tensor_tensor(
                            out=mnTres[k][:, j * 128:(j + 1) * 128],
                            in0=tp[:, k * 128:(k + 1) * 128], scalar=0.0,
                            in1=dstT.bitcast(F32),
                            op0=OP.add, op1=OP.subtract)

            # ---------------- per-tile lhs prep (fused, lookahead-1) -----
            def stage_xprep(L, i, pre):
                """Produce tile i's transposed f32r/residual/bf16 lhs splits
                and its row inv-norms. For L2 this also applies BN1+lrelu.
                Runs one iteration ahead of the sim."""
                if L == 1:
                    src = work.tile([128, MEM_DIM], F32, tag="ld", name="ld2", bufs=3)
                    nc.sync.dma_start(src, x_d[i * 128:(i + 1) * 128, :])
                else:
                    if i < len(pre):
                        hsl = pre[i]
                    else:
                        hsl = work.tile([128, MEM_DIM], F32, tag="ld",
                                        name="ld3", bufs=3)
                        nc.scalar.dma_start(hsl,
                                            h_dram[1][i * 128:(i + 1) * 128, :])
                    ya = work.tile([128, MEM_DIM], F32, tag="ya", name="ya", bufs=2)
                    nc.gpsimd.tensor_mul(ya, hsl, a2b)
                    yb = work.tile([128, MEM_DIM], F32, tag="yb", name="yb", bufs=2)
                    nc.gpsimd.tensor_add(yb, ya, b2b)
                    src = work.tile([128, MEM_DIM], F32, tag="ylr", name="ylr",
                                    bufs=2)
                    nc.scalar.activation(src, yb, AF.Lrelu, alpha=LEAKY)
                # row norms -> invn via exp(-0.5 ln)
                sq = work.tile([128, MEM_DIM], F32, tag="sqt", name="sqt2", bufs=2)
                ns = work.tile([128, 1], F32, tag="ns", name="ns", bufs=2)
                nc.scalar.activation(sq, src, AF.Square, accum_out=ns)
                lns = work.tile([128, 1], F32, tag="lns", name="lns", bufs=2)
                nc.scalar.activation(lns, ns, AF.Ln)
                invn = work.tile([128, 1], F32, tag="invn", name="invn", bufs=3)
                nc.scalar.activation(invn, lns, AF.Exp, scale=-0.5)
                # transpose + splits
                tpx = psum_tp.tile([128, MEM_DIM], F32, tag="tp")
                for k in range(2):
                    PE.transpose(tpx[:, k * 128:(k + 1) * 128],
                                 src[:, k * 128:(k + 1) * 128], ident)
                xr = work.tile([128, MEM_DIM], F32R, tag="xr", name="xr", bufs=2)
                nc.scalar.copy(xr, tpx)
                rsd = work.tile([128, MEM_DIM], F32, tag="rsdl", name="rsdl", bufs=2)
                nc.vector.scalar_tensor_tensor(
                    out=rsd, in0=tpx, scalar=0.0, in1=xr.bitcast(F32),
                    op0=OP.add, op1=OP.subtract)
                xs = work.tile([128, MEM_DIM], F32R, tag="xs", name="xs", bufs=2)
                nc.gpsimd.tensor_copy(xs, rsd)
                xb = work.tile([128, MEM_DIM], BF16, tag="xb", name="xb", bufs=2)
                nc.gpsimd.tensor_copy(xb, xr.bitcast(F32))
                return dict(xr=xr, xs=xs, xb=xb, invn=invn)

            # ---------------- main loop pieces ----------------
            def stage_sim(L, i, lhs):
                """3-pass f32r sim matmul into one 4-bank PSUM tile, drained
                in per-bank chunks."""
                xr, xs, xb = lhs["xr"], lhs["xs"], lhs["xb"]
                q = psum_q.tile([128, MEM_SIZE], F32, tag="q")
                for c in range(4):
                    sl = slice(c * 512, (c + 1) * 512)
                    for k in range(2):
                        PE.matmul(q[:, sl], xr[:, k * 128:(k + 1) * 128],
                                  mnT_r[k][:, sl], start=(k == 0), stop=False)
                    for k in range(2):
                        PE.matmul(q[:, sl], xs[:, k * 128:(k + 1) * 128],
                                  mnT_r[k][:, sl], start=False, stop=False)
                    for k in range(2):
                        PE.matmul(q[:, sl], xb[:, k * 128:(k + 1) * 128],
                                  mnTres[k][:, sl], start=False, stop=(k == 1))
                # L1: late chunks drain on DVE (feeds its own max8, keeps the
                # ACT queue clear at the iteration boundary). L2: DVE is
                # saturated by the scans+stt, all four on ACT.
                s_sb = work.tile([128, MEM_SIZE], F32, tag="s_sb", name="s_sb", bufs=2)
                for c in range(4):
                    sl = slice(c * 512, (c + 1) * 512)
                    if L == 1 and c >= 2:
                        nc.vector.tensor_copy(s_sb[:, sl], q[:, sl])
                    else:
                        nc.scalar.copy(s_sb[:, sl], q[:, sl])
                return s_sb

            def stage_scan(L, i, s_sb, invn):
                """Exact 10th-largest threshold (3 DVE passes) + exp bias."""
                m8a = work.tile([128, 8], F32, tag="m8a", name="m8a", bufs=2)
                nc.vector.max(out=m8a, in_=s_sb)
                sz = work.tile([128, MEM_SIZE], F32, tag="sz", name="sz", bufs=2)
                nc.vector.match_replace(out=sz, in_to_replace=m8a,
                                        in_values=s_sb, imm_value=NEG_BIG)
                m8b = work.tile([128, 8], F32, tag="m8b", name="m8b", bufs=2)
                nc.vector.max(out=m8b, in_=sz)
                t_ap = m8b[:, K_TOP - 8 - 1:K_TOP - 8]  # 10th largest
                negt = work.tile([128, 1], F32, tag="negt", name="negt", bufs=2)
                nc.gpsimd.tensor_scalar(negt, t_ap, -1.0, None, op0=OP.mult)
                negb = work.tile([128, 1], F32, tag="negb", name="negb", bufs=2)
                nc.gpsimd.tensor_mul(negb, negt, invn)
                return sz, m8b, negb

            def stage_weights(L, i, s_sb, sz, m8b, negb, invn):
                """exp weights, top-10 mask, Z (emitted early the next
                iteration when all inputs are long ready)."""
                t_ap = m8b[:, K_TOP - 8 - 1:K_TOP - 8]
                Z = work.tile([128, 1], F32, tag="Z", name="Z", bufs=2)
                # e written into sz (dead after its max8)
                nc.scalar.activation(sz, s_sb, AF.Exp, bias=negb, scale=invn)
                u_dt = F32 if L == 1 else BF16
                U = work.tile([128, MEM_SIZE], u_dt, tag="U", name="U", bufs=2)
                nc.vector.scalar_tensor_tensor(
                    out=U, in0=s_sb, scalar=t_ap, in1=sz,
                    op0=OP.is_ge, op1=OP.mult, accum_out=Z)
                rz = work.tile([128, 1], F32, tag="rz", name="rz", bufs=2)
                nc.vector.reciprocal(rz, Z)
                return dict(U=U, rz=rz)

            def stage2(L, i, st):
                """U transposes + h = (U/Z) @ mem + Pool stats accumulate."""
                U, rz = st["U"], st["rz"]
                ut_dt = F32 if L == 1 else BF16
                idn = ident if L == 1 else ident_b
                utp = work.tile([128, MEM_SIZE], ut_dt, tag="utp", name="utp", bufs=1)
                for g in range(4):
                    tp = psum_tp.tile([128, 512], ut_dt, tag="tp")
                    for c4 in range(4):
                        c = g * 4 + c4
                        PE.transpose(tp[:, c4 * 128:(c4 + 1) * 128],
                                     U[:, c * 128:(c + 1) * 128], idn)
                    if L == 1 and g == 3:
                        nc.vector.tensor_copy(utp[:, g * 512:(g + 1) * 512], tp)
                    else:
                        nc.scalar.copy(utp[:, g * 512:(g + 1) * 512], tp)
                mraw = mraw1 if L == 1 else mraw2
                hp = psum_h_pool.tile([128, MEM_DIM], F32, tag="hp")
                for c in range(NJ):
                    PE.matmul(hp, utp[:, c * 128:(c + 1) * 128],
                              mraw[:, c * MEM_DIM:(c + 1) * MEM_DIM],
                              start=(c == 0), stop=(c == NJ - 1))
                dst = work.tile([128, MEM_DIM], F32, tag="dst", name="dst", bufs=3)
                nc.scalar.mul(dst, hp, rz)
                nc.sync.dma_start(h_dram[L][i * 128:(i + 1) * 128, :], dst)
                nc.gpsimd.tensor_add(acc[:, 0:MEM_DIM], acc[:, 0:MEM_DIM], dst)
                sqh = work.tile([128, MEM_DIM], F32, tag="sqh", name="sqh", bufs=2)
                nc.gpsimd.tensor_mul(sqh, dst, dst)
                nc.gpsimd.tensor_add(acc[:, MEM_DIM:2 * MEM_DIM],
                                     acc[:, MEM_DIM:2 * MEM_DIM], sqh)

            def layer(L, pre):
                """Software-pipelined main loop.

                Iteration i emits: stage2(i-2); weights(i-1); sim(i);
                scans(i); xprep(i+1). Keeps every in-order engine queue
                free of waits on slow cross-engine chains.
                """
                nc.gpsimd.memset(acc, 0.0)
                lhs = {0: stage_xprep(L, 0, pre)}
                scanned = {}
                ready = {}
                for i in range(nt):
                    if i >= 2:
                        stage2(L, i - 2, ready.pop(i - 2))
                    if i >= 1:
                        ready[i - 1] = stage_weights(L, i - 1,
                                                     *scanned.pop(i - 1))
                    li = lhs.pop(i)
                    s_sb = stage_sim(L, i, li)
                    scanned[i] = (s_sb,) + stage_scan(L, i, s_sb, li["invn"]) \
                        + (li["invn"],)
                    if i + 1 < nt:
                        lhs[i + 1] = stage_xprep(L, i + 1, pre)
                # epilogue
                ready[nt - 1] = stage_weights(L, nt - 1, *scanned.pop(nt - 1))
                stage2(L, nt - 2, ready.pop(nt - 2))
                stage2(L, nt - 1, ready.pop(nt - 1))

            def prefetch_h(L, n_pre):
                tiles = []
                for i in range(n_pre):
                    hsl = work.tile([128, MEM_DIM], F32, tag=f"pre{i}",
                                    name=f"pre{L}_{i}", bufs=1)
                    nc.sync.dma_start(hsl, h_dram[L][i * 128:(i + 1) * 128, :])
                    tiles.append(hsl)
                return tiles

            N_PRE = 8

            def bn_start(L):
                """Reduce the Pool-accumulated stats, kick the AllReduce."""
                st_ps = psum_h_pool.tile([1, 2 * MEM_DIM], F32, tag="hp")
                PE.matmul(st_ps, ones_col, acc, start=True, stop=True)
                stats_sb = work.tile([1, 2 * MEM_DIM], F32, tag="stats",
                                     name="stats", bufs=1)
                nc.scalar.copy(stats_sb, st_ps)
                ar_in = dram.tile([1, 2 * MEM_DIM], F32, name=f"ar_in{L}")
                ar_out = dram.tile([1, 2 * MEM_DIM], F32, addr_space="Shared",
                                   name=f"ar_out{L}")
                nc.sync.dma_start(ar_in, stats_sb)
                nc.gpsimd.collective_compute(
                    "AllReduce", OP.add,
                    replica_groups=[list(range(n_cores))],
                    ins=[ar_in[:]], outs=[ar_out[:]],
                )
                gst = work.tile([1, 2 * MEM_DIM], F32, tag="gst", name="gst", bufs=1)
                nc.sync.dma_start(gst, ar_out)
                return gst

            def bn_finish(L, gst):
                """Global sums -> broadcast affine params a2b/b2b."""
                gamma_sb, beta_sb = gb[L]
                ab = work.tile([1, 2 * MEM_DIM], F32, tag="ab", name="ab", bufs=1)
                a_ap, b_ap = ab[:, 0:MEM_DIM], ab[:, MEM_DIM:2 * MEM_DIM]
                mu = work.tile([1, MEM_DIM], F32, tag="mu", name="mu", bufs=1)
                nc.vector.tensor_scalar(mu, gst[:, 0:MEM_DIM], 1.0 / n_total,
                                        None, op0=OP.mult)
                ex2 = work.tile([1, MEM_DIM], F32, tag="ex2", name="ex2", bufs=1)
                nc.vector.tensor_scalar(ex2, gst[:, MEM_DIM:2 * MEM_DIM],
                                        1.0 / n_total, None, op0=OP.mult)
                musq = work.tile([1, MEM_DIM], F32, tag="musq", name="musq", bufs=1)
                nc.scalar.activation(musq, mu, AF.Square)
                var = work.tile([1, MEM_DIM], F32, tag="var", name="var", bufs=1)
                nc.vector.tensor_sub(var, ex2, musq)
                # isd = exp(-0.5 ln(var + eps)) - no sqrt, no table switch
                lv = work.tile([1, MEM_DIM], F32, tag="lv", name="lv", bufs=1)
                nc.scalar.activation(lv, var, AF.Ln, bias=epsap)
                isd = work.tile([1, MEM_DIM], F32, tag="isd", name="isd", bufs=1)
                nc.scalar.activation(isd, lv, AF.Exp, scale=-0.5)
                nc.vector.tensor_mul(a_ap, gamma_sb, isd)
                mua = work.tile([1, MEM_DIM], F32, tag="mua", name="mua", bufs=1)
                nc.vector.tensor_mul(mua, mu, a_ap)
                nc.vector.tensor_sub(b_ap, beta_sb, mua)
                bc = psum_tp.tile([128, 2 * MEM_DIM], F32, tag="tp")
                PE.matmul(bc, ones_row, ab, start=True, stop=True)
                nc.scalar.copy(a2b, bc[:, 0:MEM_DIM])
                nc.scalar.copy(b2b, bc[:, MEM_DIM:2 * MEM_DIM])

            def final_out(pre):
                for i in range(nt):
                    if i < len(pre):
                        hsl = pre[i]
                    else:
                        hsl = work.tile([128, MEM_DIM], F32, tag="ld",
                                        name="ld4", bufs=3)
                        nc.scalar.dma_start(hsl,
                                            h_dram[2][i * 128:(i + 1) * 128, :])
                    eng = nc.gpsimd if i % 2 == 0 else nc.vector
                    ya = work.tile([128, MEM_DIM], F32, tag="ya", name="ya2", bufs=2)
                    eng.tensor_mul(ya, hsl, a2b)
                    yb = work.tile([128, MEM_DIM], F32, tag="yb", name="yb2", bufs=2)
                    eng.tensor_add(yb, ya, b2b)
                    yo = work.tile([128, MEM_DIM], F32, tag="ylr", name="yo", bufs=2)
                    nc.scalar.activation(yo, yb, AF.Lrelu, alpha=LEAKY)
                    nc.sync.dma_start(out_d[i * 128:(i + 1) * 128, :], yo)

            # ---------------- program ----------------
            prep_bank_a(1)
            prep_bank_b(1)
            layer(1, [])
            pre1 = prefetch_h(1, N_PRE)
            prep_bank_a(2)     # L2 bank prep overlaps the L1 pipeline
            prep_bank_b(2)     # drain-down and the AllReduce latency
            gst1 = bn_start(1)
            bn_finish(1, gst1)
            layer(2, pre1)
            pre2 = prefetch_h(2, N_PRE)
            gst2 = bn_start(2)
            bn_finish(2, gst2)
            final_out(pre2)

    nc.compile()
    return nc


_CACHE = {}


def _get_nc(n_cores, rows_per_core):
    key = (n_cores, rows_per_core)
    if key not in _CACHE:
        _CACHE[key] = build_nc(n_cores, rows_per_core)
    return _CACHE[key]


def kernel(x, mem1, mem2, gamma1, beta1, gamma2, beta2, _trace=False,
           _use_f32r=True, _n_cores=8):
    n_cores = _n_cores
    n, d = x.shape
    rows_per_core = n // n_cores
    nc = _get_nc(n_cores, rows_per_core)

    in_maps = []
    for c in range(n_cores):
        in_maps.append({
            "x": np.ascontiguousarray(x[c * rows_per_core:(c + 1) * rows_per_core]),
            "mem1": np.ascontiguousarray(mem1),
            "mem2": np.ascontiguousarray(mem2),
            "gamma1": np.ascontiguousarray(gamma1.reshape(1, -1)),
            "beta1": np.ascontiguousarray(beta1.reshape(1, -1)),
            "gamma2": np.ascontiguousarray(gamma2.reshape(1, -1)),
            "beta2": np.ascontiguousarray(beta2.reshape(1, -1)),
        })
    res = run_bass_kernel_spmd(nc, in_maps, list(range(n_cores)), trace=_trace)
    out = np.concatenate([res.results[c]["out"] for c in range(n_cores)], axis=0)
    if _trace:
        return out, res
    return out


# revision 39
# speedup vs baseline: 1.0302x; 1.0040x over previous
"""Trainium2 Bass kernel for nn_CMmodel (retrieval_knn).

Model (per layer, x2):
    sim = cosine(x, mem)                       # [N, 2048]
    S, I = top_k(sim, 10); w = softmax(relu(S))
    h = sum_k w[n,k] * mem[I[n,k]]             # [N, 256]
    h = leaky_relu(batchnorm(h))               # batch stats over ALL N rows

Strategy (8 cores, data-parallel over N):
  - Shard x rows 8 ways; replicate mem banks + BN params; AllReduce the BN
    batch statistics so the global batch stats are exact.
  - Selection must be f32-exact: near-ties between rank 10 and 11 pick
    different mem rows (measured: 1-pass f32r sim flips 0.52% of rows ->
    2.6e-2 rel err; 3-pass flips none). So sim = 3-pass f32r
    (r(x)@r(m^) + res(x)@r(m^) + bf16(x)@bf16res(m^)), threshold and mask
    computed on the f32 q values.
  - Scale-free threshold: q = x @ m^T (mem rows normalized, x NOT
    normalized). Selection is invariant to the positive 1/|x| scale; the
    softmax temperature is folded into the exp activation via
    out = Exp(q*invn - t*invn) with per-partition scale/bias.
  - L1 h-matmul must be full f32 (f32r -> 2.6e-2 rel err via L2 selection
    flips); L2 h-matmul is bf16 (final rel err 2.4e-3, verified in numpy
    simulation of this exact pipeline on the fixed-seed data).
  - NO sqrt anywhere: every 1/sqrt becomes Exp(-0.5*Ln(.)) - Ln, Exp,
    Square, Copy and parametric_relu all live in ONE ACT table
    (natural_log_exp_and_others), so there are zero mid-kernel table
    reloads (the old Sqrt<->Exp ping-pong cost 202us in the original
    baseline) AND the x/y1 norm work can fuse into the main loops.
  - Everything is ONE software-pipelined loop per layer: per-tile lhs
    prep (DMA, BN-apply for L2, transpose, f32r splits, row norms) runs
    one iteration ahead; the threshold scans run one iteration behind
    the sim; exp/mask one more behind; transpose+h-matmul two behind.
    Engine assignment is balanced per layer (L1 is PE-bound, L2 is
    PE/DVE/ACT-balanced) and in-order queues never wait on a slow
    cross-engine chain.
  - BN batch stats accumulated per-tile on the Pool engine, reduced with
    ONE ones-matmul per layer, AllReduced across cores.
"""
import sys

sys.path.insert(0, "/opt/trn_rl_repo")

import numpy as np

import concourse.bacc as bacc
import concourse.mybir as mybir
import concourse.tile as tile
from concourse.bass_utils import run_bass_kernel_spmd
from concourse.masks import make_identity
from concourse.tile import add_dep_helper

F32 = mybir.dt.float32
F32R = mybir.dt.float32r
BF16 = mybir.dt.bfloat16
AF = mybir.ActivationFunctionType
OP = mybir.AluOpType

MEM_DIM = 256
MEM_SIZE = 2048
K_TOP = 10
BN_EPS = 1e-5
LEAKY = 0.01

NJ = MEM_SIZE // 128  # 16 mem-row chunks
NEG_BIG = -1e30


def build_nc(n_cores: int, rows_per_core: int):
    nt = rows_per_core // 128
    n_total = rows_per_core * n_cores
    nc = bacc.Bacc("TRN2", target_bir_lowering=False, debug=False,
                   num_devices=n_cores)

    x_d = nc.dram_tensor("x", [rows_per_core, MEM_DIM], F32, kind="ExternalInput")
    mem_d = {
        1: nc.dram_tensor("mem1", [MEM_SIZE, MEM_DIM], F32, kind="ExternalInput"),
        2: nc.dram_tensor("mem2", [MEM_SIZE, MEM_DIM], F32, kind="ExternalInput"),
    }
    gam_d = {
        1: nc.dram_tensor("gamma1", [1, MEM_DIM], F32, kind="ExternalInput"),
        2: nc.dram_tensor("gamma2", [1, MEM_DIM], F32, kind="ExternalInput"),
    }
    bet_d = {
        1: nc.dram_tensor("beta1", [1, MEM_DIM], F32, kind="ExternalInput"),
        2: nc.dram_tensor("beta2", [1, MEM_DIM], F32, kind="ExternalInput"),
    }
    out_d = nc.dram_tensor("out", [rows_per_core, MEM_DIM], F32, kind="ExternalOutput")
    h_dram = {
        1: nc.dram_tensor("h1buf", [rows_per_core, MEM_DIM], F32),
        2: nc.dram_tensor("h2buf", [rows_per_core, MEM_DIM], F32),
    }

    with tile.TileContext(nc) as tc:
        with tc.tile_pool(name="consts", bufs=1) as consts, \
             tc.tile_pool(name="banks", bufs=1) as banks, \
             tc.tile_pool(name="work", bufs=1) as work, \
             tc.tile_pool(name="psum_q", bufs=1, space="PSUM") as psum_q, \
             tc.tile_pool(name="psum_tp", bufs=3, space="PSUM") as psum_tp, \
             tc.tile_pool(name="psum_h", bufs=1, space="PSUM") as psum_h_pool, \
             tc.tile_pool(name="dram", bufs=1, space="DRAM") as dram:

            # PE emission-order chain: accumulation groups must stay
            # contiguous on PE (interleaved matmuls drop accumulates).
            class _PEChain:
                def __init__(self):
                    self.last = None

                def _chain(self, binst):
                    if self.last is not None:
                        add_dep_helper(binst.ins, self.last.ins, sync=False,
                                       reason="pe-order")
                    self.last = binst
                    return binst

                def matmul(self, *a, **kw):
                    return self._chain(nc.tensor.matmul(*a, **kw))

                def transpose(self, *a, **kw):
                    return self._chain(nc.tensor.transpose(*a, **kw))

            PE = _PEChain()

            # ---------------- constants ----------------
            ident = consts.tile([128, 128], F32)
            make_identity(nc, ident)
            ident_b = consts.tile([128, 128], BF16, name="ident_b")
            nc.vector.tensor_copy(ident_b, ident)
            ones_col = consts.tile([128, 1], F32)
            nc.vector.memset(ones_col, 1.0)
            ones_row = consts.tile([1, 128], F32)
            nc.vector.memset(ones_row, 1.0)
            epsap = consts.tile([1, 1], F32)
            nc.vector.memset(epsap, BN_EPS)

            gb = {}
            for L in (1, 2):
                g = consts.tile([1, MEM_DIM], F32, name=f"gamma_sb{L}")
                b = consts.tile([1, MEM_DIM], F32, name=f"beta_sb{L}")
                nc.sync.dma_start(g, gam_d[L][:])
                nc.sync.dma_start(b, bet_d[L][:])
                gb[L] = (g, b)

            # persistent stores
            mnT_r = [banks.tile([128, MEM_SIZE], F32R, name=f"mnTr{k}")
                     for k in range(2)]
            mnTres = [banks.tile([128, MEM_SIZE], BF16, name=f"mnTres{k}")
                      for k in range(2)]
            mraw1 = banks.tile([128, NJ * MEM_DIM], F32, name="mraw1")
            mraw2 = banks.tile([128, NJ * MEM_DIM], BF16, name="mraw2")
            nrm2_all = consts.tile([128, NJ], F32, name="nrm2_all")
            # BN broadcast affine params (row layout)
            a2b = consts.tile([128, MEM_DIM], F32, name="a2b")
            b2b = consts.tile([128, MEM_DIM], F32, name="b2b")
            # BN stats accumulator: cols 0:256 sum(h), 256:512 sum(h^2)
            acc = consts.tile([128, 2 * MEM_DIM], F32, name="acc")

            # ---------------- mem bank prep ----------------
            def prep_bank_a(L):
                """Load raw mem (f32 for L1 h-matmul, bf16 for L2) + row
                norms^2. DMAs ride the ACT queue so they never sit behind a
                collective-blocked Sync DMA."""
                md = mem_d[L]
                for j in range(NJ):
                    mr = work.tile([128, MEM_DIM], F32, tag="ld", name="ld", bufs=3)
                    nc.scalar.dma_start(mr, md[j * 128:(j + 1) * 128, :])
                    if L == 1:
                        nc.vector.tensor_copy(
                            mraw1[:, j * MEM_DIM:(j + 1) * MEM_DIM], mr)
                    else:
                        nc.vector.tensor_copy(
                            mraw2[:, j * MEM_DIM:(j + 1) * MEM_DIM], mr)
                    sq = work.tile([128, MEM_DIM], F32, tag="sqt", name="sqt", bufs=2)
                    nc.scalar.activation(sq, mr, AF.Square,
                                         accum_out=nrm2_all[:, j:j + 1])

            def prep_bank_b(L):
                """inm = exp(-0.5 ln(|m|^2)); normalize/transpose/split."""
                md = mem_d[L]
                lnm = work.tile([128, NJ], F32, tag="lnm", name="lnm", bufs=1)
                nc.scalar.activation(lnm, nrm2_all, AF.Ln)
                inm0 = work.tile([128, NJ], F32, tag="inm0", name="inm0", bufs=1)
                nc.scalar.activation(inm0, lnm, AF.Exp, scale=-0.5)
                # one Newton step: the 1e-5 Ln/Exp error on mem norms scales
                # all of a row's sims together and flips ~0.1% of top-10
                # selections; Newton brings it to ~1e-9 (selection-safe)
                t1 = work.tile([128, NJ], F32, tag="nt1", name="nt1", bufs=1)
                nc.vector.tensor_mul(t1, inm0, inm0)
                nc.vector.tensor_mul(t1, t1, nrm2_all)
                nc.vector.tensor_scalar(t1, t1, -0.5, 1.5, op0=OP.mult, op1=OP.add)
                inm = work.tile([128, NJ], F32, tag="inm", name="inm", bufs=1)
                nc.vector.tensor_mul(inm, inm0, t1)
                for j in range(NJ):
                    mr2 = work.tile([128, MEM_DIM], F32, tag="ld", name="ld5", bufs=3)
                    nc.scalar.dma_start(mr2, md[j * 128:(j + 1) * 128, :])
                    mnsc = work.tile([128, MEM_DIM], F32, tag="mnsc", name="mnsc",
                                     bufs=2)
                    nc.scalar.mul(mnsc, mr2, inm[:, j:j + 1])
                    tp = psum_tp.tile([128, MEM_DIM], F32, tag="tp")
                    for k in range(2):
                        PE.transpose(tp[:, k * 128:(k + 1) * 128],
                                     mnsc[:, k * 128:(k + 1) * 128], ident)
                    for k in range(2):
                        dstT = mnT_r[k][:, j * 128:(j + 1) * 128]
                        nc.scalar.copy(dstT, tp[:, k * 128:(k + 1) * 128])
                        nc.vector.scalar_tensor_tensor(
                            out=mnTres[k][:, j * 128:(j + 1) * 128],
                            in0=tp[:, k * 128:(k + 1) * 128], scalar=0.0,
                            in1=dstT.bitcast(F32),
                            op0=OP.add, op1=OP.subtract)

            # ---------------- per-tile lhs prep (fused, lookahead-1) -----
            def stage_xprep(L, i, pre):
                """Produce tile i's transposed f32r/residual/bf16 lhs splits
                and its row inv-norms. For L2 this also applies BN1+lrelu.
                Runs one iteration ahead of the sim."""
                if L == 1:
                    src = work.tile([128, MEM_DIM], F32, tag="ld", name="ld2", bufs=3)
                    nc.sync.dma_start(src, x_d[i * 128:(i + 1) * 128, :])
                else:
                    if i < len(pre):
                        hsl = pre[i]
                    else:
                        hsl = work.tile([128, MEM_DIM], F32, tag="ld",
                                        name="ld3", bufs=3)
                        nc.scalar.dma_start(hsl,
                                            h_dram[1][i * 128:(i + 1) * 128, :])
                    ya = work.tile([128, MEM_DIM], F32, tag="ya", name="ya", bufs=2)
                    nc.gpsimd.tensor_mul(ya, hsl, a2b)
                    yb = work.tile([128, MEM_DIM], F32, tag="yb", name="yb", bufs=2)
                    nc.gpsimd.tensor_add(yb, ya, b2b)
                    src = work.tile([128, MEM_DIM], F32, tag="ylr", name="ylr",
                                    bufs=2)
                    nc.scalar.activation(src, yb, AF.Lrelu, alpha=LEAKY)
                # row norms -> invn via exp(-0.5 ln)
                sq = work.tile([128, MEM_DIM], F32, tag="sqt", name="sqt2", bufs=2)
                ns = work.tile([128, 1], F32, tag="ns", name="ns", bufs=2)
                nc.scalar.activation(sq, src, AF.Square, accum_out=ns)
                lns = work.tile([128, 1], F32, tag="lns", name="lns", bufs=2)
                nc.scalar.activation(lns, ns, AF.Ln)
                invn = work.tile([128, 1], F32, tag="invn", name="invn", bufs=3)
                nc.scalar.activation(invn, lns, AF.Exp, scale=-0.5)
                # transpose + splits
                tpx = psum_tp.tile([128, MEM_DIM], F32, tag="tp")
                for k in range(2):
                    PE.transpose(tpx[:, k * 128:(k + 1) * 128],
                                 src[:, k * 128:(k + 1) * 128], ident)
                xr = work.tile([128, MEM_DIM], F32R, tag="xr", name="xr", bufs=2)
                nc.scalar.copy(xr, tpx)
                rsd = work.tile([128, MEM_DIM], F32, tag="rsdl", name="rsdl", bufs=2)
                nc.vector.scalar_tensor_tensor(
                    out=rsd, in0=tpx, scalar=0.0, in1=xr.bitcast(F32),
                    op0=OP.add, op1=OP.subtract)
                xs = work.tile([128, MEM_DIM], F32R, tag="xs", name="xs", bufs=2)
                nc.gpsimd.tensor_copy(xs, rsd)
                xb = work.tile([128, MEM_DIM], BF16, tag="xb", name="xb", bufs=2)
                nc.gpsimd.tensor_copy(xb, xr.bitcast(F32))
                return dict(xr=xr, xs=xs, xb=xb, invn=invn)

            # ---------------- main loop pieces ----------------
            def stage_sim(L, i, lhs):
                """3-pass f32r sim matmul into one 4-bank PSUM tile, drained
                in per-bank chunks."""
                xr, xs, xb = lhs["xr"], lhs["xs"], lhs["xb"]
                q = psum_q.tile([128, MEM_SIZE], F32, tag="q")
                for c in range(4):
                    sl = slice(c * 512, (c + 1) * 512)
                    for k in range(2):
                        PE.matmul(q[:, sl], xr[:, k * 128:(k + 1) * 128],
                                  mnT_r[k][:, sl], start=(k == 0), stop=False)
                    for k in range(2):
                        PE.matmul(q[:, sl], xs[:, k * 128:(k + 1) * 128],
                                  mnT_r[k][:, sl], start=False, stop=False)
                    for k in range(2):
                        PE.matmul(q[:, sl], xb[:, k * 128:(k + 1) * 128],
                                  mnTres[k][:, sl], start=False, stop=(k == 1))
                # L1: late chunks drain on DVE (feeds its own max8, keeps the
                # ACT queue clear at the iteration boundary). L2: DVE is
                # saturated by the scans+stt, all four on ACT.
                s_sb = work.tile([128, MEM_SIZE], F32, tag="s_sb", name="s_sb", bufs=2)
                for c in range(4):
                    sl = slice(c * 512, (c + 1) * 512)
                    if L == 1 and c >= 2:
                        nc.vector.tensor_copy(s_sb[:, sl], q[:, sl])
                    else:
                        nc.scalar.copy(s_sb[:, sl], q[:, sl])
                return s_sb

            def stage_scan(L, i, s_sb, invn):
                """Exact 10th-largest threshold (3 DVE passes) + exp bias."""
                m8a = work.tile([128, 8], F32, tag="m8a", name="m8a", bufs=2)
                nc.vector.max(out=m8a, in_=s_sb)
                sz = work.tile([128, MEM_SIZE], F32, tag="sz", name="sz", bufs=2)
                nc.vector.match_replace(out=sz, in_to_replace=m8a,
                                        in_values=s_sb, imm_value=NEG_BIG)
                m8b = work.tile([128, 8], F32, tag="m8b", name="m8b", bufs=2)
                nc.vector.max(out=m8b, in_=sz)
                t_ap = m8b[:, K_TOP - 8 - 1:K_TOP - 8]  # 10th largest
                negt = work.tile([128, 1], F32, tag="negt", name="negt", bufs=2)
                nc.gpsimd.tensor_scalar(negt, t_ap, -1.0, None, op0=OP.mult)
                negb = work.tile([128, 1], F32, tag="negb", name="negb", bufs=2)
                nc.gpsimd.tensor_mul(negb, negt, invn)
                return sz, m8b, negb

            def stage_weights(L, i, s_sb, sz, m8b, negb, invn):
                """exp weights, top-10 mask, Z (emitted early the next
                iteration when all inputs are long ready)."""
                t_ap = m8b[:, K_TOP - 8 - 1:K_TOP - 8]
                Z = work.tile([128, 1], F32, tag="Z", name="Z", bufs=2)
                # e written into sz (dead after its max8)
                nc.scalar.activation(sz, s_sb, AF.Exp, bias=negb, scale=invn)
                u_dt = F32 if L == 1 else BF16
                U = work.tile([128, MEM_SIZE], u_dt, tag="U", name="U", bufs=2)
                nc.vector.scalar_tensor_tensor(
                    out=U, in0=s_sb, scalar=t_ap, in1=sz,
                    op0=OP.is_ge, op1=OP.mult, accum_out=Z)
                rz = work.tile([128, 1], F32, tag="rz", name="rz", bufs=2)
                nc.vector.reciprocal(rz, Z)
                return dict(U=U, rz=rz)

            def stage2(L, i, st):
                """U transposes + h = (U/Z) @ mem + Pool stats accumulate."""
                U, rz = st["U"], st["rz"]
                ut_dt = F32 if L == 1 else BF16
                idn = ident if L == 1 else ident_b
                utp = work.tile([128, MEM_SIZE], ut_dt, tag="utp", name="utp", bufs=1)
                for g in range(4):
                    tp = psum_tp.tile([128, 512], ut_dt, tag="tp")
                    for c4 in range(4):
                        c = g * 4 + c4
                        PE.transpose(tp[:, c4 * 128:(c4 + 1) * 128],
                                     U[:, c * 128:(c + 1) * 128], idn)
                    if L == 1 and g == 3:
                        nc.vector.tensor_copy(utp[:, g * 512:(g + 1) * 512], tp)
                    else:
                        nc.scalar.copy(utp[:, g * 512:(g + 1) * 512], tp)
                mraw = mraw1 if L == 1 else mraw2
                hp = psum_h_pool.tile([128, MEM_DIM], F32, tag="hp")
                for c in range(NJ):
                    PE.matmul(hp, utp[:, c * 128:(c + 1) * 128],
                              mraw[:, c * MEM_DIM:(c + 1) * MEM_DIM],
                              start=(c == 0), stop=(c == NJ - 1))
                dst = work.tile([128, MEM_DIM], F32, tag="dst", name="dst", bufs=3)
                nc.scalar.mul(dst, hp, rz)
                nc.sync.dma_start(h_dram[L][i * 128:(i + 1) * 128, :], dst)
                nc.gpsimd.tensor_add(acc[:, 0:MEM_DIM], acc[:, 0:MEM_DIM], dst)
                sqh = work.tile([128, MEM_DIM], F32, tag="sqh", name="sqh", bufs=2)
                nc.gpsimd.tensor_mul(sqh, dst, dst)
                nc.gpsimd.tensor_add(acc[:, MEM_DIM:2 * MEM_DIM],
                                     acc[:, MEM_DIM:2 * MEM_DIM], sqh)

            def layer(L, pre):
                """Software-pipelined main loop.

                Iteration i emits: stage2(i-2); weights(i-1); sim(i);
                scans(i); xprep(i+1). Keeps every in-order engine queue
                free of waits on slow cross-engine chains.
                """
                nc.gpsimd.memset(acc, 0.0)
                lhs = {0: stage_xprep(L, 0, pre)}
                scanned = {}
                ready = {}
                for i in range(nt):
                    if i >= 2:
                        stage2(L, i - 2, ready.pop(i - 2))
                    if i >= 1:
                        ready[i - 1] = stage_weights(L, i - 1,
                                                     *scanned.pop(i - 1))
                    li = lhs.pop(i)
                    s_sb = stage_sim(L, i, li)
                    scanned[i] = (s_sb,) + stage_scan(L, i, s_sb, li["invn"]) \
                        + (li["invn"],)
                    if i + 1 < nt:
                        lhs[i + 1] = stage_xprep(L, i + 1, pre)
                # epilogue
                ready[nt - 1] = stage_weights(L, nt - 1, *scanned.pop(nt - 1))
                stage2(L, nt - 2, ready.pop(nt - 2))
                stage2(L, nt - 1, ready.pop(nt - 1))

            def prefetch_h(L, n_pre):
                tiles = []
                for i in range(n_pre):
                    hsl = work.tile([128, MEM_DIM], F32, tag=f"pre{i}",
                                    name=f"pre{L}_{i}", bufs=1)
                    nc.sync.dma_start(hsl, h_dram[L][i * 128:(i + 1) * 128, :])
                    tiles.append(hsl)
                return tiles

            N_PRE = 8

            def bn_start(L):
                """Reduce the Pool-accumulated stats, kick the AllReduce."""
                st_ps = psum_h_pool.tile([1, 2 * MEM_DIM], F32, tag="hp")
                PE.matmul(st_ps, ones_col, acc, start=True, stop=True)
                stats_sb = work.tile([1, 2 * MEM_DIM], F32, tag="stats",
                                     name="stats", bufs=1)
                nc.scalar.copy(stats_sb, st_ps)
                ar_in = dram.tile([1, 2 * MEM_DIM], F32, name=f"ar_in{L}")
                ar_out = dram.tile([1, 2 * MEM_DIM], F32, addr_space="Shared",
                                   name=f"ar_out{L}")
                nc.sync.dma_start(ar_in, stats_sb)
                nc.gpsimd.collective_compute(
                    "AllReduce", OP.add,
                    replica_groups=[list(range(n_cores))],
                    ins=[ar_in[:]], outs=[ar_out[:]],
                )
                gst = work.tile([1, 2 * MEM_DIM], F32, tag="gst", name="gst", bufs=1)
                nc.sync.dma_start(gst, ar_out)
                return gst

            def bn_finish(L, gst):
                """Global sums -> broadcast affine params a2b/b2b."""
                gamma_sb, beta_sb = gb[L]
                ab = work.tile([1, 2 * MEM_DIM], F32, tag="ab", name="ab", bufs=1)
                a_ap, b_ap = ab[:, 0:MEM_DIM], ab[:, MEM_DIM:2 * MEM_DIM]
                mu = work.tile([1, MEM_DIM], F32, tag="mu", name="mu", bufs=1)
                nc.vector.tensor_scalar(mu, gst[:, 0:MEM_DIM], 1.0 / n_total,
                                        None, op0=OP.mult)
                ex2 = work.tile([1, MEM_DIM], F32, tag="ex2", name="ex2", bufs=1)
                nc.vector.tensor_scalar(ex2, gst[:, MEM_DIM:2 * MEM_DIM],
                                        1.0 / n_total, None, op0=OP.mult)
                musq = work.tile([1, MEM_DIM], F32, tag="musq", name="musq", bufs=1)
                nc.scalar.activation(musq, mu, AF.Square)
                var = work.tile([1, MEM_DIM], F32, tag="var", name="var", bufs=1)
                nc.vector.tensor_sub(var, ex2, musq)
                # isd = exp(-0.5 ln(var + eps)) - no sqrt, no table switch
                lv = work.tile([1, MEM_DIM], F32, tag="lv", name="lv", bufs=1)
                nc.scalar.activation(lv, var, AF.Ln, bias=epsap)
                isd = work.tile([1, MEM_DIM], F32, tag="isd", name="isd", bufs=1)
                nc.scalar.activation(isd, lv, AF.Exp, scale=-0.5)
                nc.vector.tensor_mul(a_ap, gamma_sb, isd)
                mua = work.tile([1, MEM_DIM], F32, tag="mua", name="mua", bufs=1)
                nc.vector.tensor_mul(mua, mu, a_ap)
                nc.vector.tensor_sub(b_ap, beta_sb, mua)
                bc = psum_tp.tile([128, 2 * MEM_DIM], F32, tag="tp")
                PE.matmul(bc, ones_row, ab, start=True, stop=True)
                nc.scalar.copy(a2b, bc[:, 0:MEM_DIM])
                nc.scalar.copy(b2b, bc[:, MEM_DIM:2 * MEM_DIM])

            def final_out(pre):
                for i in range(nt):
                    if i < len(pre):
                        hsl = pre[i]
                    else:
                        hsl = work.tile([128, MEM_DIM], F32, tag="ld",
                                        name="ld4", bufs=3)
                        nc.scalar.dma_start(hsl,
                                            h_dram[2][i * 128:(i + 1) * 128, :])
                    eng = nc.gpsimd if i % 2 == 0 else nc.vector
                    ya = work.tile([128, MEM_DIM], F32, tag="ya", name="ya2", bufs=2)
                    eng.tensor_mul(ya, hsl, a2b)
                    yb = work.tile([128, MEM_DIM], F32, tag="yb", name="yb2", bufs=2)
                    eng.tensor_add(yb, ya, b2b)
                    yo = work.tile([128, MEM_DIM], F32, tag="ylr", name="yo", bufs=2)
                    nc.scalar.activation(yo, yb, AF.Lrelu, alpha=LEAKY)
                    nc.sync.dma_start(out_d[i * 128:(i + 1) * 128, :], yo)

            # ---------------- program ----------------
            prep_bank_a(1)
            prep_bank_b(1)
            layer(1, [])
            pre1 = prefetch_h(1, N_PRE)
            prep_bank_a(2)     # L2 bank prep overlaps the L1 pipeline
            prep_bank_b(2)     # drain-down and the AllReduce latency
            gst1 = bn_start(1)
            bn_finish(1, gst1)
            layer(2, pre1)
            pre2 = prefetch_h(2, N_PRE)
            gst2 = bn_start(2)
            bn_finish(2, gst2)
            final_out(pre2)

    nc.compile()
    return nc


_CACHE = {}


def _get_nc(n_cores, rows_per_core):
    key = (n_cores, rows_per_core)
    if key not in _CACHE:
        _CACHE[key] = build_nc(n_cores, rows_per_core)
    return _CACHE[key]


def kernel(x, mem1, mem2, gamma1, beta1, gamma2, beta2, _trace=False,
           _use_f32r=True, _n_cores=8):
    n_cores = _n_cores
    n, d = x.shape
    rows_per_core = n // n_cores
    nc = _get_nc(n_cores, rows_per_core)

    in_maps = []
    for c in range(n_cores):
        in_maps.append({
            "x": np.ascontiguousarray(x[c * rows_per_core:(c + 1) * rows_per_core]),
            "mem1": np.ascontiguousarray(mem1),
            "mem2": np.ascontiguousarray(mem2),
            "gamma1": np.ascontiguousarray(gamma1.reshape(1, -1)),
            "beta1": np.ascontiguousarray(beta1.reshape(1, -1)),
            "gamma2": np.ascontiguousarray(gamma2.reshape(1, -1)),
            "beta2": np.ascontiguousarray(beta2.reshape(1, -1)),
        })
    res = run_bass_kernel_spmd(nc, in_maps, list(range(n_cores)), trace=_trace)
    out = np.concatenate([res.results[c]["out"] for c in range(n_cores)], axis=0)
    if _trace:
        return out, res
    return out
